# revision 1
# baseline (speedup 1.0000x reference)
"""Trainium2 Bass kernel for nn_AttentionModule (S=2048, D=4096, H=32, KV=8, HD=128).

Sharding: tensor-parallel over heads across 8 NeuronCores. Core c owns q-heads
4c..4c+3 and kv-head c (GQA groups stay intact). Each core computes RMSNorm
(norm_w folded into weights on host, rstd computed on device), its QKV
projection shard, RoPE, causal attention for its 4 heads, and a partial output
projection against its 512 columns of wo. The host sums the 8 partial outputs
(the "all-reduce" of the tensor-parallel layout).

All matmuls run as float32r (TF32-like single-pass mode, 1 cycle/row at free
dim >= 256 vs 4 cycles/row for exact fp32).

Layout notes:
 - Everything on-chip is "transposed": hT [d, s], qT/kT/vT [head_dim, s].
   Host pre-transposes hidden and the weight shards so the contraction dim is
   always the partition dim.
 - RoPE: the reference uses interleaved complex pairs (2i, 2i+1). We permute
   the head-dim rows of wq/wk on the host so pairs land at (i, i+64), turning
   RoPE into rotate-half form: q' = q*cos + (P_rot@q)*sin, computed with one
   128x128 signed-permutation matmul + 3 vector ops per tile.
 - Softmax runs in scores-transposed [t, s] layout: denominators via a
   ones-column matmul (reduction over the partition dim), reciprocal on DVE,
   broadcast back over partitions via a K=1 ones-row matmul.
 - Causal masking: full t-chunks below the diagonal need no mask; the 4
   diagonal chunks per s-block use affine_select on GPSIMD
   (iota = j - p - 128r >= 0).
 - All ACT activations (Exp, Ln, Copy) are kept inside one table set
   (natural_log_exp_and_others) to avoid ~1.3us table reloads; the Bacc
   subclass below reorders the candidate tables so that set wins.
"""
import sys

sys.path.insert(0, "/opt/trn_rl_repo")

import math
from contextlib import ExitStack

import numpy as np

import bass_rust as _bass_rust
import concourse.bacc as bacc
import concourse.mybir as mybir
import concourse.tile as tile
from concourse.bass_utils import run_bass_kernel_spmd
from concourse.hw_specs import get_activation_tables

F32R = mybir.dt.float32r
F32 = mybir.dt.float32
ALU = mybir.AluOpType
ACTF = mybir.ActivationFunctionType

S, D, H, KV, HD = 2048, 4096, 32, 8, 128
NCORES = 8
QH = H // NCORES          # 4 q heads per core
QI = QH * HD              # 512 local q dims
DC = D // 128             # 32 contraction chunks
SB = 512                  # s-block width
NSB = S // SB             # 4 s-blocks
NTC = S // 128            # 16 t-chunks
EPS = 1e-6
THETA = 50000.0
SM_SCALE = 1.0 / math.sqrt(HD)

LAST_EXEC_NS = None
LAST_RESULT = None
_CACHE = {}

# pipeline-depth knobs (tuned via timeline sim)
KNOBS = dict(hb_bufs=8, sq_act=True, t12_bufs=1, expp_bufs=3, qtmp_bufs=3,
             sc_bufs=2, wkv_bufs=3, sqp_bufs=2, hb_dc=2, interleave=True,
             mask_dve=True, csb=1, kv_dc=4, wq_dc=2, wo_cache=True,
             wop_bufs=8, obig_w=4, outb_bufs=4, early_evac=True, mask_pool_sb=1, ham_warmup=24)


class _Bacc(bacc.Bacc):
    """Bacc with activation tables reordered so the one set containing
    Exp+Ln+Copy+Square is preferred — avoids per-call ACT table reloads."""

    def insert_act_table_loads(self):
        has_activation = any(
            isinstance(i, mybir.InstActivation)
            for b in self.main_func.blocks
            for i in b.instructions
        )
        if not has_activation:
            return
        tables = list(get_activation_tables(self.m.arch).items())
        tables.sort(key=lambda kv: 0 if kv[0] == "natural_log_exp_and_others" else 1)
        _bass_rust.insert_act_table_loads(self, tables)


def _build(skip_compile=False):
    nc = bacc.Bacc("TRN2", target_bir_lowering=False, debug=False)

    hT_d = nc.dram_tensor("hT", [D, S], F32R, kind="ExternalInput")
    wqT_d = nc.dram_tensor("wqT", [D, QI], F32R, kind="ExternalInput")
    wkT_d = nc.dram_tensor("wkT", [D, HD], F32R, kind="ExternalInput")
    wvT_d = nc.dram_tensor("wvT", [D, HD], F32R, kind="ExternalInput")
    woT_d = nc.dram_tensor("woT", [QI, D], F32R, kind="ExternalInput")
    cos_d = nc.dram_tensor("cosT", [128, S], F32R, kind="ExternalInput")
    sin_d = nc.dram_tensor("sinT", [128, S], F32R, kind="ExternalInput")
    prot_d = nc.dram_tensor("protT", [128, 128], F32R, kind="ExternalInput")
    ident_d = nc.dram_tensor("ident", [128, 128], F32R, kind="ExternalInput")
    onec_d = nc.dram_tensor("ones_col", [128, 1], F32R, kind="ExternalInput")
    oner_d = nc.dram_tensor("ones_row", [1, 128], F32R, kind="ExternalInput")
    mask_d = nc.dram_tensor("maskT", [128, 4 * SB], F32R, kind="ExternalInput")
    out_d = nc.dram_tensor("outp", [S, D], F32, kind="ExternalOutput")
    if KNOBS.get("debug_dumps", False):
        dbg_q = nc.dram_tensor("dbg_q", [128, QH, S], F32, kind="ExternalOutput")
        dbg_k = nc.dram_tensor("dbg_k", [128, S], F32, kind="ExternalOutput")
        dbg_vn = nc.dram_tensor("dbg_vn", [128, NTC, HD], F32, kind="ExternalOutput")
        dbg_at = nc.dram_tensor("dbg_at", [128, QH, S], F32, kind="ExternalOutput")
        dbg_rb = nc.dram_tensor("dbg_rb", [128, NSB, SB], F32, kind="ExternalOutput")
        dbg_sq = nc.dram_tensor("dbg_sq", [128, NSB, SB], F32, kind="ExternalOutput")

    hT3 = hT_d.rearrange("(o p) s -> p o s", p=128)      # [128, 32, 2048]
    wqT3 = wqT_d.rearrange("(o p) i -> p o i", p=128)    # [128, 32, 512]
    wkT3 = wkT_d.rearrange("(o p) e -> p o e", p=128)    # [128, 32, 128]
    wvT3 = wvT_d.rearrange("(o p) e -> p o e", p=128)
    woT3 = woT_d.rearrange("(g p) j -> p g j", p=128)    # [128, 4, 4096]
    out4 = out_d.rearrange("(g p) j -> p g j", p=128)    # [128, 16, 4096]

    HB_DC = KNOBS.get("hb_dc", 2)  # hT chunks per DMA

    with tile.TileContext(nc) as tc:
        with ExitStack() as root:
            consts = root.enter_context(tc.tile_pool(name="consts", bufs=1))
            persist = root.enter_context(tc.tile_pool(name="persist", bufs=1))

            onec_t = consts.tile([128, 1], F32R, tag="onec")
            nc.sync.dma_start(out=onec_t, in_=onec_d[:, :])
            oner_t = consts.tile([1, 128], F32R, tag="oner")
            nc.sync.dma_start(out=oner_t, in_=oner_d[:, :])
            eps_t = consts.tile([1, 1], F32, tag="eps")
            nc.vector.memset(eps_t, EPS)

            qT_all = persist.tile([128, QH, S], F32R, tag="qT")
            kT_all = persist.tile([128, S], F32R, tag="kT")
            v_nat = persist.tile([128, NTC, HD], F32R, tag="vn")

            # ------------- Phase 1: QKV projections + rstd + RoPE -------------
            with ExitStack() as ph1:
                c1 = ph1.enter_context(tc.tile_pool(name="c1", bufs=1))
                cos_t = c1.tile([128, S], F32R, tag="cos")
                sin_t = c1.tile([128, S], F32R, tag="sin")
                prot_t = c1.tile([128, 128], F32R, tag="prot")
                ident_t = c1.tile([128, 128], F32R, tag="ident")
                c1_loaded = [False]

                wqp = ph1.enter_context(tc.tile_pool(name="wqp", bufs=1))
                wq_t = wqp.tile([128, DC, QI], F32R, tag="wqr")
                wkvp = ph1.enter_context(tc.tile_pool(name="wkvp", bufs=KNOBS["wkv_bufs"]))
                hb = ph1.enter_context(tc.tile_pool(name="hb", bufs=KNOBS["hb_bufs"]))
                sqp = ph1.enter_context(tc.tile_pool(name="sqp", bufs=KNOBS["sqp_bufs"]))
                scr = ph1.enter_context(tc.tile_pool(name="scr", bufs=2))
                acc_ps = ph1.enter_context(
                    tc.tile_pool(name="acc_ps", bufs=1, space="PSUM")
                )
                misc_ps = ph1.enter_context(
                    tc.tile_pool(name="misc_ps", bufs=2, space="PSUM")
                )

                if KNOBS.get("ham_warmup", 0):
                    # HAM clock-ramp warm-up: dummy matmuls on a zeroed tile
                    # during the initial DMA wait so real matmuls start at
                    # 2.4GHz (PE_HAM needs ~3.4us of activity; cost model
                    # doesn't simulate this, hardware does).
                    wu_f = scr.tile([128, SB], F32, tag="sqacc", bufs=2,
                                    name="warmup_f")
                    nc.vector.memset(wu_f, 0.0)
                    wu = scr.tile([128, SB], F32R, tag="qtmp", bufs=KNOBS["qtmp_bufs"],
                                  name="warmup_src")
                    nc.vector.tensor_copy(out=wu, in_=wu_f)
                    wu_ps = misc_ps.tile([128, SB], F32, tag="misc", name="wu_ps")
                    for _w in range(KNOBS["ham_warmup"]):
                        nc.tensor.matmul(wu_ps, wu[:, :128], wu,
                                         start=(_w == 0),
                                         stop=(_w == KNOBS["ham_warmup"] - 1))

                for sb in range(NSB):
                    ssl = slice(SB * sb, SB * (sb + 1))
                    q_ps = [
                        acc_ps.tile([128, SB], F32, tag=f"q{i}", name=f"q_ps{i}")
                        for i in range(QH)
                    ]
                    k_ps = acc_ps.tile([128, SB], F32, tag="k")
                    v_ps = acc_ps.tile([128, SB], F32, tag="v")
                    sqacc = scr.tile([128, SB], F32, tag="sqacc", bufs=2)
                    sqr = scr.tile([128, SB], F32R, tag="sqr", bufs=1)
                    KV_DC = KNOBS.get("kv_dc", 4)  # wk/wv chunk width
                    WQ_DC = KNOBS.get("wq_dc", 2)  # wq load width (sb 0)
                    for hc in range(DC // HB_DC):
                        ht2 = hb.tile([128, HB_DC, SB], F32R, tag="h")
                        nc.sync.dma_start(out=ht2, in_=hT3[:, HB_DC*hc:HB_DC*(hc+1), ssl])
                        if (HB_DC * hc) % KV_DC == 0:
                            kc0 = HB_DC * hc
                            wkc = wkvp.tile([128, KV_DC, HD], F32R, tag="wk2")
                            nc.sync.dma_start(
                                out=wkc, in_=wkT3[:, kc0:kc0+KV_DC, :])
                            wvc = wkvp.tile([128, KV_DC, HD], F32R, tag="wv2")
                            nc.sync.dma_start(
                                out=wvc, in_=wvT3[:, kc0:kc0+KV_DC, :])
                        for j in range(HB_DC):
                            dc = HB_DC * hc + j
                            ht = ht2[:, j, :]
                            if sb == 0 and dc % WQ_DC == 0:
                                nc.sync.dma_start(out=wq_t[:, dc:dc+WQ_DC, :],
                                                  in_=wqT3[:, dc:dc+WQ_DC, :])
                            wqc = wq_t[:, dc, :]
                            if sb == 0 and dc == 8 and not c1_loaded[0]:
                                nc.sync.dma_start(out=cos_t, in_=cos_d[:, :])
                                nc.sync.dma_start(out=sin_t, in_=sin_d[:, :])
                                nc.sync.dma_start(out=prot_t, in_=prot_d[:, :])
                                nc.sync.dma_start(out=ident_t, in_=ident_d[:, :])
                                c1_loaded[0] = True
                            sq = sqp.tile([128, SB], F32, tag="sq")
                            if KNOBS["sq_act"]:
                                nc.scalar.activation(out=sq, in_=ht, func=ACTF.Square)
                            else:
                                nc.vector.tensor_tensor(sq, ht, ht, ALU.mult)
                            sq_eng = nc.gpsimd if KNOBS.get("sqacc_pool", False) else nc.vector
                            if dc == 0:
                                sq_eng.tensor_copy(out=sqacc, in_=sq)
                            elif dc == DC - 1:
                                sq_eng.tensor_tensor(sqr, sqacc, sq, ALU.add)
                            else:
                                sq_eng.tensor_tensor(sqacc, sqacc, sq, ALU.add)
                            for i in range(QH):
                                nc.tensor.matmul(
                                    q_ps[i],
                                    wqc[:, 128 * i: 128 * (i + 1)],
                                    ht,
                                    start=(dc == 0),
                                    stop=(dc == DC - 1),
                                )
                            nc.tensor.matmul(
                                k_ps, wkc[:, dc % KV_DC, :], ht,
                                start=(dc == 0), stop=(dc == DC - 1),
                            )
                            nc.tensor.matmul(
                                v_ps, wvc[:, dc % KV_DC, :], ht,
                                start=(dc == 0), stop=(dc == DC - 1),
                            )
                    # rstd row for this s-block (exp(-0.5 ln(ms)) — same ACT set).
                    # PSUM evacuation is plain copies (no rstd dependency) so the
                    # next s-block's accumulation starts immediately; rstd is
                    # folded into per-block cos/sin tables instead.
                    ms_ps = misc_ps.tile([1, SB], F32, tag="misc", name="ms_ps")
                    nc.tensor.matmul(ms_ps, onec_t, sqr, start=True, stop=True)
                    lnt = scr.tile([1, SB], F32, tag="lnt", bufs=1)
                    nc.scalar.activation(
                        out=lnt, in_=ms_ps, func=ACTF.Sqrt, scale=1.0 / D, bias=eps_t
                    )
                    rstd = scr.tile([1, SB], F32R, tag="rstd", bufs=1)
                    with nc.allow_low_precision(reason="rstd row fp32r"):
                        nc.vector.reciprocal(out=rstd, in_=lnt.bitcast(F32R))
                    rb_ps = misc_ps.tile([128, SB], F32, tag="misc", name="rb_ps")
                    nc.tensor.matmul(rb_ps, oner_t, rstd, start=True, stop=True)
                    rb_sb = scr.tile([128, SB], F32R, tag="rb_sb", bufs=KNOBS.get("csb", 2))
                    nc.vector.tensor_copy(out=rb_sb, in_=rb_ps.bitcast(F32R))
                    if KNOBS.get("debug_dumps", False):
                        nc.sync.dma_start(out=dbg_rb[:, sb, :], in_=rb_sb.bitcast(F32))
                        nc.sync.dma_start(out=dbg_sq[:, sb, :], in_=sqr.bitcast(F32))
                    cosrb = scr.tile([128, SB], F32R, tag="cosrb", bufs=KNOBS.get("csb", 2))
                    nc.vector.tensor_tensor(cosrb, cos_t[:, ssl], rb_sb, ALU.mult)
                    sinrb = scr.tile([128, SB], F32R, tag="sinrb", bufs=KNOBS.get("csb", 2))
                    nc.vector.tensor_tensor(sinrb, sin_t[:, ssl], rb_sb, ALU.mult)

                    # q + rope (scale folded into cosrb/sinrb) -> qT_all
                    for i in range(QH):
                        qtmp = scr.tile([128, SB], F32R, tag="qtmp", bufs=KNOBS["qtmp_bufs"])
                        nc.vector.tensor_copy(out=qtmp, in_=q_ps[i].bitcast(F32R))
                        rot_ps = misc_ps.tile([128, SB], F32, tag="misc",
                                              name=f"rot_q{i}")
                        nc.tensor.matmul(rot_ps, prot_t, qtmp, start=True, stop=True)
                        t1 = scr.tile([128, SB], F32R, tag="t1", bufs=KNOBS["t12_bufs"])
                        nc.vector.tensor_tensor(t1, qtmp, cosrb, ALU.mult)
                        t2 = scr.tile([128, SB], F32R, tag="t2", bufs=KNOBS["t12_bufs"])
                        nc.vector.tensor_tensor(
                            t2, rot_ps.bitcast(F32R), sinrb, ALU.mult
                        )
                        (nc.gpsimd if KNOBS.get("rope_add_pool", False) else nc.vector
                         ).tensor_tensor(qT_all[:, i, ssl], t1, t2, ALU.add)
                    # k + rope -> kT_all
                    ktmp = scr.tile([128, SB], F32R, tag="qtmp", bufs=KNOBS["qtmp_bufs"], name="ktmp")
                    nc.vector.tensor_copy(out=ktmp, in_=k_ps.bitcast(F32R))
                    rot_ps = misc_ps.tile([128, SB], F32, tag="misc", name="rot_k")
                    nc.tensor.matmul(rot_ps, prot_t, ktmp, start=True, stop=True)
                    t1 = scr.tile([128, SB], F32R, tag="t1", bufs=KNOBS["t12_bufs"], name="t1k")
                    nc.vector.tensor_tensor(t1, ktmp, cosrb, ALU.mult)
                    t2 = scr.tile([128, SB], F32R, tag="t2", bufs=KNOBS["t12_bufs"], name="t2k")
                    nc.vector.tensor_tensor(
                        t2, rot_ps.bitcast(F32R), sinrb, ALU.mult
                    )
                    nc.vector.tensor_tensor(kT_all[:, ssl], t1, t2, ALU.add)
                    # v: evacuate, scale by rstd, transpose to v_nat
                    vtmp = scr.tile([128, SB], F32R, tag="qtmp", bufs=KNOBS["qtmp_bufs"], name="vtmp")
                    nc.vector.tensor_copy(out=vtmp, in_=v_ps.bitcast(F32R))
                    vsc = scr.tile([128, SB], F32R, tag="vsc", bufs=KNOBS.get("csb", 2))
                    nc.vector.tensor_tensor(vsc, vtmp, rb_sb, ALU.mult)
                    for j in range(SB // 128):
                        tcx = (SB // 128) * sb + j
                        vtr_ps = misc_ps.tile([128, 128], F32R, tag="misc",
                                              name=f"vtr{tcx}")
                        nc.tensor.transpose(
                            vtr_ps, vsc[:, 128 * j: 128 * (j + 1)], ident_t
                        )
                        nc.vector.tensor_copy(out=v_nat[:, tcx, :], in_=vtr_ps)

            # attnT allocated only now (frees phase-1 SBUF for resident wq)
            persist2 = root.enter_context(tc.tile_pool(name="persist2", bufs=1))
            attnT = persist2.tile([128, QH, S], F32R, tag="attnT")
            mask_t = persist2.tile([128, 4, SB], F32R, tag="mask")
            nc.sync.dma_start(out=mask_t, in_=mask_d.rearrange("p (r s) -> p r s", s=SB))

            # phase-4 pools allocated first so they get PSUM banks / SBUF
            # disjoint from phase 3 (enables clean overlap)
            o_ps_p = root.enter_context(tc.tile_pool(name="o_ps", bufs=2, space="PSUM"))
            outb = root.enter_context(tc.tile_pool(name="outb", bufs=KNOBS.get("outb_bufs", 2)))
            wop = root.enter_context(tc.tile_pool(name="wop", bufs=KNOBS.get("wop_bufs", 2)))

            # ------------- Phase 3+4 interleaved ------------------------------
            ph3 = ExitStack()
            sc_ps_p = ph3.enter_context(
                tc.tile_pool(name="sc_ps", bufs=KNOBS["sc_bufs"], space="PSUM")
            )
            att_ps_p = ph3.enter_context(
                tc.tile_pool(name="att_ps", bufs=KNOBS.get("att_bufs", 1), space="PSUM")
            )
            sum_ps_p = ph3.enter_context(
                tc.tile_pool(name="sum_ps", bufs=1, space="PSUM")
            )
            expp = ph3.enter_context(tc.tile_pool(name="expp", bufs=KNOBS["expp_bufs"]))
            scr3 = ph3.enter_context(tc.tile_pool(name="scr3", bufs=2))

            def emit_attention(sb):
                for h in range(QH):
                    ssl = slice(SB * sb, SB * (sb + 1))
                    n_tc = (SB // 128) * (sb + 1)
                    att_ps = att_ps_p.tile([128, SB], F32, tag="att",
                                           name=f"att{h}_{sb}")
                    if KNOBS.get("sums_dve", False):
                        eacc = scr3.tile([128, SB], F32R, tag="eacc", bufs=2,
                                         name=f"eacc{h}_{sb}")
                        eaccr = eacc
                    else:
                        sum_ps = sum_ps_p.tile([1, SB], F32, tag="sumrc",
                                               name=f"sum{h}_{sb}")
                    SCP = 2 if KNOBS.get("sc_pair", True) else 1
                    for tp in range(n_tc // SCP):
                        # paired scores tiles -> one wide exp
                        sc_ps = sc_ps_p.tile([128, SCP, SB], F32, tag="sc",
                                             name=f"sc{h}_{sb}_{tp}")
                        e_pair = expp.tile([128, SCP, SB], F32R, tag="e",
                                           name=f"e{h}_{sb}_{tp}")
                        for u in range(SCP):
                            tcx = SCP * tp + u
                            nc.tensor.matmul(
                                sc_ps[:, u, :],
                                kT_all[:, 128 * tcx: 128 * (tcx + 1)],
                                qT_all[:, h, ssl],
                                start=True, stop=True,
                            )
                        nc.scalar.activation(
                            out=e_pair, in_=sc_ps, func=ACTF.Exp, scale=SM_SCALE
                        )
                        for u in range(SCP):
                            tcx = SCP * tp + u
                            e_sb = e_pair[:, u, :]
                            r = tcx - (SB // 128) * sb
                            if r >= 0:
                                # diagonal chunk: zero where t > s; dense early
                                # blocks go to idle GPSIMD, late ones to DVE
                                if sb <= KNOBS.get("mask_pool_sb", -1):
                                    nc.gpsimd.affine_select(
                                        e_sb, e_sb,
                                        pattern=[[1, SB]],
                                        compare_op=ALU.is_ge,
                                        fill=0.0,
                                        base=-(128 * r),
                                        channel_multiplier=-1,
                                    )
                                else:
                                    nc.vector.tensor_tensor(
                                        e_sb, e_sb, mask_t[:, r, :], ALU.mult
                                    )
                            nc.tensor.matmul(
                                att_ps, v_nat[:, tcx, :], e_sb,
                                start=(tcx == 0), stop=(tcx == n_tc - 1),
                            )
                            if KNOBS.get("sums_dve", False):
                                if tcx == 0:
                                    nc.vector.tensor_copy(out=eacc, in_=e_sb)
                                elif tcx == n_tc - 1:
                                    nc.vector.tensor_tensor(eaccr, eacc, e_sb, ALU.add)
                                else:
                                    nc.vector.tensor_tensor(eacc, eacc, e_sb, ALU.add)
                            else:
                                nc.tensor.matmul(
                                    sum_ps, onec_t, e_sb,
                                    start=(tcx == 0), stop=(tcx == n_tc - 1),
                                )
                    # evacuate att bank immediately (unnormalized), then
                    # normalize attnT in place once the recip row is ready —
                    # frees the single att PSUM bank ~2us earlier for head h+1
                    if KNOBS.get("early_evac", True):
                        nc.vector.tensor_copy(
                            out=attnT[:, h, ssl], in_=att_ps.bitcast(F32R)
                        )
                    if KNOBS.get("sums_dve", False):
                        sum_ps = sum_ps_p.tile([1, SB], F32, tag="sumrc",
                                               name=f"sum{h}_{sb}")
                        nc.tensor.matmul(sum_ps, onec_t, eaccr, start=True, stop=True)
                    rcv = scr3.tile([1, SB], F32R, tag="rcv", bufs=2,
                                    name=f"rcv{h}_{sb}")
                    with nc.allow_low_precision(reason="softmax recip row"):
                        nc.vector.reciprocal(out=rcv, in_=sum_ps.bitcast(F32R))
                    rc_ps = sum_ps_p.tile([128, SB], F32, tag="sumrc",
                                          name=f"rc{h}_{sb}")
                    nc.tensor.matmul(rc_ps, oner_t, rcv, start=True, stop=True)
                    rc_sb = scr3.tile([128, SB], F32R, tag="rcsb", bufs=2,
                                      name=f"rcsb{h}_{sb}")
                    nc.vector.tensor_copy(out=rc_sb, in_=rc_ps.bitcast(F32R))
                    if KNOBS.get("early_evac", True):
                        nc.vector.tensor_tensor(
                            attnT[:, h, ssl], attnT[:, h, ssl], rc_sb, ALU.mult
                        )
                    else:
                        nc.vector.tensor_tensor(
                            attnT[:, h, ssl], att_ps.bitcast(F32R), rc_sb, ALU.mult
                        )

            woc_cache = {}
            o_holder = [o_ps_p]

            def emit_outproj(g):
                OBW = KNOBS.get("obig_w", 8)  # sc-tiles per out staging/DMA
                for jt in range(D // SB):
                    jsl = slice(SB * jt, SB * (jt + 1))
                    if KNOBS.get("wo_cache", False):
                        if g == 0:
                            woc = wop.tile([128, QH, SB], F32R, tag="wo",
                                           name=f"wo{jt}")
                            (nc.scalar if KNOBS.get("out_actq", False) else nc.sync
                             ).dma_start(out=woc, in_=woT3[:, :, jsl])
                            woc_cache[jt] = woc
                        woc = woc_cache[jt]
                    else:
                        woc = wop.tile([128, QH, SB], F32R, tag="wo",
                                       name=f"wo{jt}_{g}")
                        (nc.scalar if KNOBS.get("out_actq", False) else nc.sync
                         ).dma_start(out=woc, in_=woT3[:, :, jsl])
                    for q in range(8 // OBW):
                        o_big = outb.tile([128, OBW, SB], F32, tag="obig",
                                          name=f"ob{jt}_{g}_{q}")
                        for si in range(OBW):
                            sc = 8 * g + OBW * q + si
                            o_ps = o_holder[0].tile([128, SB], F32, tag="o",
                                               name=f"o{jt}_{sc}")
                            for h in range(QH):
                                nc.tensor.matmul(
                                    o_ps,
                                    attnT[:, h, 128 * sc: 128 * (sc + 1)],
                                    woc[:, h, :],
                                    start=(h == 0), stop=(h == QH - 1),
                                )
                            if si % 2 == 0:
                                nc.vector.tensor_copy(out=o_big[:, si, :], in_=o_ps)
                            else:
                                nc.scalar.copy(out=o_big[:, si, :], in_=o_ps)
                        g0 = 8 * g + OBW * q
                        (nc.scalar if KNOBS.get("out_actq", False) else nc.sync
                         ).dma_start(
                            out=out4[:, g0: g0 + OBW, jsl], in_=o_big
                        )

            if KNOBS.get("debug_dumps", False):
                nc.sync.dma_start(out=dbg_q[:, :, :], in_=qT_all.bitcast(F32))
                nc.sync.dma_start(out=dbg_k[:, :], in_=kT_all.bitcast(F32))
                nc.sync.dma_start(out=dbg_vn[:, :, :], in_=v_nat.bitcast(F32))
            if KNOBS.get("interleave", True):
                emit_attention(0)
                emit_attention(1)
                emit_outproj(0)   # sc 0..7 only needs attnT of sb 0-1
                emit_attention(2)
                emit_attention(3)
                if KNOBS.get("g1_deep", False):
                    ph3.close()  # release attention PSUM banks for g1
                    o2 = root.enter_context(
                        tc.tile_pool(name="o_ps2", bufs=KNOBS.get("o2_bufs", 4),
                                     space="PSUM"))
                    o_holder[0] = o2
                emit_outproj(1)
                if not KNOBS.get("g1_deep", False):
                    ph3.close()
                if KNOBS.get("debug_dumps", False):
                    nc.sync.dma_start(out=dbg_at[:, :, :], in_=attnT.bitcast(F32))
            else:
                for _sb in range(NSB):
                    emit_attention(_sb)
                emit_outproj(0)
                emit_outproj(1)
                ph3.close()

    if not skip_compile:
        nc.compile()
    return nc


def _host_prep(inputs):
    """Build per-core input maps (shard + transpose + fold norm_w + rope-perm)."""
    hidden = np.ascontiguousarray(np.asarray(inputs["hidden"], dtype=np.float32))
    norm_w = np.asarray(inputs["norm_w"], dtype=np.float32)
    wq = np.asarray(inputs["wq"], dtype=np.float32)
    wk = np.asarray(inputs["wk"], dtype=np.float32)
    wv = np.asarray(inputs["wv"], dtype=np.float32)
    wo = np.asarray(inputs["wo"], dtype=np.float32)

    perm = np.concatenate([np.arange(0, HD, 2), np.arange(1, HD, 2)])
    # RoPE tables exactly as the reference builds them
    freqs = 1.0 / THETA ** (np.arange(0, HD, 2)[: HD // 2].astype(np.float32) / HD)
    ang = np.outer(np.arange(S), freqs).astype(np.float32)   # [S, 64]
    cosT = np.ascontiguousarray(
        np.concatenate([np.cos(ang).T, np.cos(ang).T], axis=0).astype(np.float32)
    )
    sinT = np.ascontiguousarray(
        np.concatenate([np.sin(ang).T, np.sin(ang).T], axis=0).astype(np.float32)
    )
    Pr = np.zeros((HD, HD), np.float32)
    Pr[np.arange(64), np.arange(64) + 64] = -1.0
    Pr[np.arange(64) + 64, np.arange(64)] = 1.0
    protT = np.ascontiguousarray(Pr.T)

    hT = np.ascontiguousarray(hidden.T)
    ident = np.eye(128, dtype=np.float32)
    # diagonal causal masks: maskT[p, r*512 + c] = 1 if 128*r + p <= c else 0
    p_i = np.arange(128)[:, None]
    c_i = np.arange(SB)[None, :]
    maskT = np.concatenate(
        [(128 * r + p_i <= c_i).astype(np.float32) for r in range(4)], axis=1
    )
    maskT = np.ascontiguousarray(maskT)
    ones_col = np.ones((128, 1), np.float32)
    ones_row = np.ones((1, 128), np.float32)

    in_maps = []
    for c in range(NCORES):
        wq_c = wq[QI * c: QI * (c + 1)].reshape(QH, HD, D)[:, perm, :].reshape(QI, D)
        wqT = np.ascontiguousarray((wq_c * norm_w[None, :]).T)
        wk_c = wk[HD * c: HD * (c + 1)][perm, :]
        wkT = np.ascontiguousarray((wk_c * norm_w[None, :]).T)
        wv_c = wv[HD * c: HD * (c + 1)]
        wvT = np.ascontiguousarray((wv_c * norm_w[None, :]).T)
        woT = np.ascontiguousarray(wo[:, QI * c: QI * (c + 1)].T)
        in_maps.append({
            "hT": hT, "wqT": wqT, "wkT": wkT, "wvT": wvT, "woT": woT,
            "cosT": cosT, "sinT": sinT, "protT": protT, "ident": ident,
            "ones_col": ones_col, "ones_row": ones_row, "maskT": maskT,
        })
    return in_maps


def kernel(**inputs) -> np.ndarray:
    global LAST_EXEC_NS, LAST_RESULT
    if "nc" not in _CACHE:
        _CACHE["nc"] = _build()
    nc = _CACHE["nc"]
    in_maps = _host_prep(inputs)
    res = run_bass_kernel_spmd(nc, in_maps, core_ids=list(range(NCORES)))
    LAST_RESULT = res
    LAST_EXEC_NS = res.exec_time_ns
    out = res.results[0]["outp"].astype(np.float32).copy()
    for c in range(1, NCORES):
        out += res.results[c]["outp"]
    return out



# revision 30
# speedup vs baseline: 1.2220x; 1.2220x over previous
"""Trainium2 Bass kernel for nn_AttentionModule (S=2048, D=4096, H=32, KV=8, HD=128).

Sharding: tensor-parallel over heads across 8 NeuronCores. Core c owns q-heads
4c..4c+3 and kv-head c (GQA groups stay intact). Each core computes RMSNorm
(norm_w folded into weights on host, rstd computed on device), its QKV
projection shard, RoPE, causal attention for its 4 heads, and a partial output
projection against its 512 columns of wo. The host sums the 8 partial outputs
(the "all-reduce" of the tensor-parallel layout).

v3: whole pipeline in bf16 (PSUM accumulation fp32; rstd / softmax-denominator
math fp32). bf16 matmuls cost 1 cycle/row like fp32r but halve DMA + SBUF and
unlock DVE 2x perf modes. Exp row-sums run as DVE bf16 chain adds + one
ones-matmul per (head, s-block). rstd = exp(-0.5 ln(ms)) so every ACT function
(Square/Ln/Exp/Copy) lives in one table set — no mid-kernel table reloads.

Scheduling: each s-block's rstd/RoPE/evacuation tail is emitted *inside* the
next s-block's matmul stream (phase 1) or between the first attention heads
(last block), so the in-order PE queue always has dense matmul work while the
serial ACT/DVE chains resolve. Attention head finalization (sum -> recip ->
broadcast -> normalize) lags one head; output projection is emitted per
128-row s-chunk interleaved between the next s-block's attention heads.
Evacuation copies spread across ACT/DVE/Pool.

Causal handling: scores matmuls for the 4 diagonal t-chunks per s-block are
trimmed to the live column range (bf16 matmuls have no <256-free-dim penalty);
the masked multiply covers the full range and zeroes the stale left part. The
exp for the (r2,r3) diagonal pair is likewise trimmed to columns 256: for
s-blocks >= 1 (for block 0 the e buffers must be written full-width once so
uninitialized SBUF never reaches the mask-multiply).
"""
import sys

sys.path.insert(0, "/opt/trn_rl_repo")

import math
from contextlib import ExitStack

import numpy as np

import bass_rust as _bass_rust
import concourse.bacc as bacc
import concourse.mybir as mybir
import concourse.tile as tile
from concourse.bass_utils import run_bass_kernel_spmd
from concourse.hw_specs import get_activation_tables

F32R = mybir.dt.float32r
F32 = mybir.dt.float32
BF16 = mybir.dt.bfloat16
ALU = mybir.AluOpType
ACTF = mybir.ActivationFunctionType

S, D, H, KV, HD = 2048, 4096, 32, 8, 128
NCORES = 8
QH = H // NCORES          # 4 q heads per core
QI = QH * HD              # 512 local q dims
DC = D // 128             # 32 contraction chunks
SB = 512                  # s-block width
NSB = S // SB             # 4 s-blocks
NTC = S // 128            # 16 t-chunks
EPS = 1e-6
THETA = 50000.0
SM_SCALE = 1.0 / math.sqrt(HD)

LAST_EXEC_NS = None
LAST_RESULT = None
_CACHE = {}

KNOBS = dict(hb_bufs=7, t12_bufs=1, expp_bufs=3, qtmp_bufs=6,
             sc_bufs=4, wkv_bufs=2, sqp_bufs=3, hb_dc=2, kv_dc=4, wq_dc=4,
             wop_bufs=8, outb_bufs=2, o_ps_bufs=2, eacc_bufs=2,
             mask_pool_sb=-1, ham_warmup=7, csb=1,
             score_trim=True, exp_trim=True,
             norm_pool=False, evac_act=True, obig_dve=True)

import os as _os
if _os.environ.get("KNOBS_JSON"):
    import json as _json
    KNOBS.update(_json.loads(_os.environ["KNOBS_JSON"]))


class _Bacc(bacc.Bacc):
    """Bacc with activation tables reordered so the one set containing
    Exp+Ln+Copy+Square is preferred."""

    def insert_act_table_loads(self):
        has_activation = any(
            isinstance(i, mybir.InstActivation)
            for b in self.main_func.blocks
            for i in b.instructions
        )
        if not has_activation:
            return
        tables = list(get_activation_tables(self.m.arch).items())
        tables.sort(key=lambda kv: 0 if kv[0] == "natural_log_exp_and_others" else 1)
        _bass_rust.insert_act_table_loads(self, tables)


def _build(skip_compile=False):
    nc = bacc.Bacc("TRN2", target_bir_lowering=False, debug=False)

    hT_d = nc.dram_tensor("hT", [D, S], BF16, kind="ExternalInput")
    wqT_d = nc.dram_tensor("wqT", [D, QI], BF16, kind="ExternalInput")
    wkT_d = nc.dram_tensor("wkT", [D, HD], BF16, kind="ExternalInput")
    wvT_d = nc.dram_tensor("wvT", [D, HD], BF16, kind="ExternalInput")
    woT_d = nc.dram_tensor("woT", [QI, D], BF16, kind="ExternalInput")
    cos_d = nc.dram_tensor("cosT", [128, S], F32R, kind="ExternalInput")
    sin_d = nc.dram_tensor("sinT", [128, S], F32R, kind="ExternalInput")
    prot_d = nc.dram_tensor("protT", [128, 128], F32R, kind="ExternalInput")
    ident_d = nc.dram_tensor("ident", [128, 128], F32R, kind="ExternalInput")
    onec_d = nc.dram_tensor("ones_col", [128, 1], F32R, kind="ExternalInput")
    onecb_d = nc.dram_tensor("ones_col_bf", [128, 1], BF16, kind="ExternalInput")
    oner_d = nc.dram_tensor("ones_row", [1, 128], F32R, kind="ExternalInput")
    mask_d = nc.dram_tensor("maskT", [128, 4 * SB], BF16, kind="ExternalInput")
    out_d = nc.dram_tensor("outp", [S, D], BF16, kind="ExternalOutput")

    hT3 = hT_d.rearrange("(o p) s -> p o s", p=128)      # [128, 32, 2048]
    wqT3 = wqT_d.rearrange("(o p) i -> p o i", p=128)    # [128, 32, 512]
    wkT3 = wkT_d.rearrange("(o p) e -> p o e", p=128)    # [128, 32, 128]
    wvT3 = wvT_d.rearrange("(o p) e -> p o e", p=128)
    woT3 = woT_d.rearrange("(g p) j -> p g j", p=128)    # [128, 4, 4096]
    out4 = out_d.rearrange("(g p) j -> p g j", p=128)    # [128, 16, 4096]

    HB_DC = KNOBS.get("hb_dc", 2)  # hT chunks per DMA

    with tile.TileContext(nc) as tc:
        with ExitStack() as root:
            consts = root.enter_context(tc.tile_pool(name="consts", bufs=1))
            persist = root.enter_context(tc.tile_pool(name="persist", bufs=1))

            onec_t = consts.tile([128, 1], F32R, tag="onec")
            nc.sync.dma_start(out=onec_t, in_=onec_d[:, :])
            onecb_t = consts.tile([128, 1], BF16, tag="onecb")
            nc.sync.dma_start(out=onecb_t, in_=onecb_d[:, :])
            oner_t = consts.tile([1, 128], F32R, tag="oner")
            nc.sync.dma_start(out=oner_t, in_=oner_d[:, :])
            eps_t = consts.tile([1, 1], F32, tag="eps")
            nc.vector.memset(eps_t, EPS)

            qT_all = persist.tile([128, QH, S], BF16, tag="qT")
            kT_all = persist.tile([128, S], BF16, tag="kT")
            v_nat = persist.tile([128, NTC, HD], BF16, tag="vn")
            attnT = persist.tile([128, QH, S], BF16, tag="attnT")
            mask_t = persist.tile([128, 4, SB], BF16, tag="mask")

            # phase-3/4 SBUF staging allocated up front (fits alongside
            # phase 1; lets wo/mask DMAs run during phase-1 DMA idle)
            outb = root.enter_context(tc.tile_pool(name="outb", bufs=KNOBS.get("outb_bufs", 3)))
            wop = root.enter_context(tc.tile_pool(name="wop", bufs=KNOBS.get("wop_bufs", 8)))
            woc_cache = {}

            def load_woc(jt):
                woc = wop.tile([128, QH, SB], BF16, tag="wo", name=f"wo{jt}")
                nc.sync.dma_start(out=woc, in_=woT3[:, :, SB * jt: SB * (jt + 1)])
                woc_cache[jt] = woc

            # ------------- Phase 1: QKV projections + rstd + RoPE -------------
            # c1/scr live on root: the sb3 tail (emitted between the first
            # attention heads) still needs them in phase 3
            c1 = root.enter_context(tc.tile_pool(name="c1", bufs=1))
            cos_t = c1.tile([128, S], F32R, tag="cos")
            sin_t = c1.tile([128, S], F32R, tag="sin")
            prot_t = c1.tile([128, 128], F32R, tag="prot")
            ident_t = c1.tile([128, 128], F32R, tag="ident")
            c1_loaded = [False]

            scr = root.enter_context(tc.tile_pool(name="scr", bufs=2))
            ph1 = ExitStack()
            wqp = ph1.enter_context(tc.tile_pool(name="wqp", bufs=1))
            wq_t = wqp.tile([128, DC, QI], BF16, tag="wqr")
            wkvp = ph1.enter_context(tc.tile_pool(name="wkvp", bufs=KNOBS["wkv_bufs"]))
            hb = ph1.enter_context(tc.tile_pool(name="hb", bufs=KNOBS["hb_bufs"]))
            sqp = ph1.enter_context(tc.tile_pool(name="sqp", bufs=KNOBS["sqp_bufs"]))
            acc_ps = ph1.enter_context(tc.tile_pool(name="acc_ps", bufs=1, space="PSUM"))
            misc_ps = ph1.enter_context(tc.tile_pool(name="misc_ps", bufs=2, space="PSUM"))
            # tail closures allocate their PSUM through this holder: misc_ps
            # during phase 1, the ph3 tail pool for the last block
            tail_psum = [misc_ps]

            if KNOBS.get("ham_warmup", 0):
                # HAM clock-ramp warm-up: dummy matmuls during initial DMA
                # wait so real matmuls start at 2.4GHz.
                wu_f = scr.tile([128, SB], F32, tag="wuf", bufs=1,
                                name="warmup_f")
                nc.vector.memset(wu_f, 0.0)
                wu = scr.tile([128, SB], F32R, tag="qtmp", bufs=KNOBS["qtmp_bufs"],
                              name="warmup_src")
                with nc.allow_low_precision(reason="warmup zeros"):
                    nc.vector.tensor_copy(out=wu, in_=wu_f)
                wu_ps = misc_ps.tile([128, SB], F32, tag="misc", name="wu_ps")
                for _w in range(KNOBS["ham_warmup"]):
                    nc.tensor.matmul(wu_ps, wu[:, :128], wu,
                                     start=(_w == 0),
                                     stop=(_w == KNOBS["ham_warmup"] - 1))

            def make_tail(sb, q_ps, k_ps, v_ps, sqacc, sqacc2):
                """rstd + rope + v-transpose for s-block sb, split into
                closures emitted later (inside the next block's matmul
                stream). Returns a list of emit-closures.

                For the last block the six accumulator PSUMs are evacuated to
                SBUF right here (ACT copies, no PE involvement) so the
                attention pools can take over every PSUM bank; the deferred
                closures then run RoPE from the SBUF temps."""
                ssl = slice(SB * sb, SB * (sb + 1))
                parts = []
                last = sb == NSB - 1
                tmps = {}
                if last:
                    for nm, ps in [("q0", q_ps[0]), ("q1", q_ps[1]),
                                   ("q2", q_ps[2]), ("q3", q_ps[3]),
                                   ("k", k_ps), ("v", v_ps)]:
                        t = scr.tile([128, SB], F32R, tag="qtmp",
                                     bufs=KNOBS["qtmp_bufs"], name=f"fin_{nm}")
                        with nc.allow_low_precision(reason="rope tmp f32r"):
                            nc.scalar.copy(out=t, in_=ps)
                        tmps[nm] = t

                def p_rstd():
                    # ms = colsum(sqacc) + colsum(sqacc2) via two accumulating
                    # K=1 matmuls; rstd = exp(-0.5 ln(ms/D + eps))
                    ms_ps = tail_psum[0].tile([1, SB], F32, tag="misc", name=f"ms{sb}")
                    nc.tensor.matmul(ms_ps, onec_t, sqacc,
                                     start=True, stop=False)
                    nc.tensor.matmul(ms_ps, onec_t, sqacc2,
                                     start=False, stop=True)
                    lnt = scr.tile([1, SB], F32, tag="lnt", bufs=1, name=f"ln{sb}")
                    nc.scalar.activation(
                        out=lnt, in_=ms_ps, func=ACTF.Sqrt, scale=1.0 / D, bias=eps_t
                    )
                    rstd = scr.tile([1, SB], F32R, tag="rstd", bufs=1, name=f"rst{sb}")
                    with nc.allow_low_precision(reason="rstd row fp32r"):
                        nc.vector.reciprocal(out=rstd, in_=lnt.bitcast(F32R))
                    rb_ps = tail_psum[0].tile([128, SB], F32, tag="misc", name=f"rb{sb}")
                    nc.tensor.matmul(rb_ps, oner_t, rstd,
                                     start=True, stop=True)
                    rb_sb = scr.tile([128, SB], F32R, tag="rb_sb",
                                     bufs=KNOBS.get("csb", 2), name=f"rbs{sb}")
                    nc.vector.tensor_copy(out=rb_sb, in_=rb_ps.bitcast(F32R))
                    cosrb = scr.tile([128, SB], F32R, tag="cosrb",
                                     bufs=KNOBS.get("csb", 2), name=f"cr{sb}")
                    nc.vector.tensor_tensor(cosrb, cos_t[:, ssl], rb_sb, ALU.mult)
                    sinrb = scr.tile([128, SB], F32R, tag="sinrb",
                                     bufs=KNOBS.get("csb", 2), name=f"sr{sb}")
                    nc.vector.tensor_tensor(sinrb, sin_t[:, ssl], rb_sb, ALU.mult)
                    parts.append((rb_sb, cosrb, sinrb))

                def rope_one(dst, src_ps, cosrb, sinrb, nm, tmp=None):
                    # evac on ACT (f32r); rot on PE; t1/t2/add DVE in f32r
                    # (homogeneous dtypes, baseline-proven on hw); the add
                    # writes the bf16 destination directly
                    if tmp is None:
                        tmp = scr.tile([128, SB], F32R, tag="qtmp",
                                       bufs=KNOBS["qtmp_bufs"], name=f"tmp{nm}")
                        with nc.allow_low_precision(reason="rope tmp f32r"):
                            nc.scalar.copy(out=tmp, in_=src_ps)
                    rot_ps = tail_psum[0].tile([128, SB], F32, tag="misc",
                                               name=f"rot{nm}")
                    nc.tensor.matmul(rot_ps, prot_t, tmp, start=True, stop=True)
                    t1 = scr.tile([128, SB], F32R, tag="t1",
                                  bufs=KNOBS["t12_bufs"], name=f"t1{nm}")
                    nc.vector.tensor_tensor(t1, tmp, cosrb, ALU.mult)
                    t2 = scr.tile([128, SB], F32R, tag="t2",
                                  bufs=KNOBS["t12_bufs"], name=f"t2{nm}")
                    nc.vector.tensor_tensor(t2, rot_ps.bitcast(F32R), sinrb, ALU.mult)
                    nc.vector.tensor_tensor(dst, t1, t2, ALU.add)

                def p_k():
                    rb_sb, cosrb, sinrb = parts[0]
                    rope_one(kT_all[:, ssl], k_ps, cosrb, sinrb, f"k{sb}",
                             tmp=tmps.get("k"))

                def p_q(i):
                    def f():
                        rb_sb, cosrb, sinrb = parts[0]
                        rope_one(qT_all[:, i, ssl], q_ps[i], cosrb, sinrb,
                                 f"q{i}_{sb}", tmp=tmps.get(f"q{i}"))
                    return f

                def p_v():
                    rb_sb, cosrb, sinrb = parts[0]
                    vtmp = tmps.get("v")
                    if vtmp is None:
                        vtmp = scr.tile([128, SB], F32R, tag="qtmp",
                                        bufs=KNOBS["qtmp_bufs"], name=f"vtmp{sb}")
                        with nc.allow_low_precision(reason="rope tmp f32r"):
                            nc.scalar.copy(out=vtmp, in_=v_ps)
                    vsc = scr.tile([128, SB], F32R, tag="vsc",
                                   bufs=KNOBS.get("csb", 2), name=f"vsc{sb}")
                    nc.vector.tensor_tensor(vsc, vtmp, rb_sb, ALU.mult)
                    for j in range(SB // 128):
                        tcx = (SB // 128) * sb + j
                        vtr_ps = tail_psum[0].tile([128, 128], F32R, tag="misc",
                                                   name=f"vtr{tcx}")
                        nc.tensor.transpose(
                            vtr_ps, vsc[:, 128 * j: 128 * (j + 1)], ident_t
                        )
                        nc.vector.tensor_copy(out=v_nat[:, tcx, :], in_=vtr_ps)

                return [p_rstd, p_k, p_q(0), p_q(1), p_q(2), p_q(3), p_v]

            pending_tail = []

            for sb in range(NSB):
                ssl = slice(SB * sb, SB * (sb + 1))
                q_ps = [
                    acc_ps.tile([128, SB], F32, tag=f"q{i}", name=f"q_ps{i}")
                    for i in range(QH)
                ]
                k_ps = acc_ps.tile([128, SB], F32, tag="k")
                v_ps = acc_ps.tile([128, SB], F32, tag="v")
                sqacc = scr.tile([128, SB], F32R, tag="sqacc", bufs=2)
                sqacc2 = scr.tile([128, SB], F32R, tag="sqacc2", bufs=2)
                KV_DC = KNOBS.get("kv_dc", 4)  # wk/wv chunk width
                WQ_DC = KNOBS.get("wq_dc", 2)  # wq load width (sb 0)
                for hc in range(DC // HB_DC):
                    ht2 = hb.tile([128, HB_DC, SB], BF16, tag="h")
                    nc.sync.dma_start(out=ht2, in_=hT3[:, HB_DC*hc:HB_DC*(hc+1), ssl])
                    if (HB_DC * hc) % KV_DC == 0:
                        kc0 = HB_DC * hc
                        wkc = wkvp.tile([128, KV_DC, HD], BF16, tag="wk2")
                        nc.sync.dma_start(out=wkc, in_=wkT3[:, kc0:kc0+KV_DC, :])
                        wvc = wkvp.tile([128, KV_DC, HD], BF16, tag="wv2")
                        nc.sync.dma_start(out=wvc, in_=wvT3[:, kc0:kc0+KV_DC, :])
                    for j in range(HB_DC):
                        dc = HB_DC * hc + j
                        ht = ht2[:, j, :]
                        if sb == 0 and dc % WQ_DC == 0:
                            nc.sync.dma_start(out=wq_t[:, dc:dc+WQ_DC, :],
                                              in_=wqT3[:, dc:dc+WQ_DC, :])
                        wqc = wq_t[:, dc, :]
                        if sb == 0 and dc == 8 and not c1_loaded[0]:
                            nc.sync.dma_start(out=cos_t, in_=cos_d[:, :])
                            nc.sync.dma_start(out=sin_t, in_=sin_d[:, :])
                            nc.sync.dma_start(out=prot_t, in_=prot_d[:, :])
                            nc.sync.dma_start(out=ident_t, in_=ident_d[:, :])
                            nc.sync.dma_start(
                                out=mask_t,
                                in_=mask_d.rearrange("p (r s) -> p r s", s=SB))
                            c1_loaded[0] = True
                        if sb >= 2 and dc % 8 == 4:
                            jt = 4 * (sb - 2) + dc // 8
                            load_woc(jt)
                        sq = sqp.tile([128, SB], F32, tag="sq")
                        nc.scalar.activation(out=sq, in_=ht, func=ACTF.Square)
                        # two interleaved f32 accumulation chains:
                        # even dc on DVE, odd dc on GPSIMD
                        acc, eng = ((sqacc, nc.vector) if dc % 2 == 0
                                    else (sqacc2, nc.gpsimd))
                        if dc < 2:
                            eng.tensor_copy(out=acc, in_=sq)
                        else:
                            eng.tensor_tensor(acc, acc, sq, ALU.add)
                        for i in range(QH):
                            nc.tensor.matmul(
                                q_ps[i],
                                wqc[:, 128 * i: 128 * (i + 1)],
                                ht,
                                start=(dc == 0),
                                stop=(dc == DC - 1),
                            )
                        nc.tensor.matmul(
                            k_ps, wkc[:, dc % KV_DC, :], ht,
                            start=(dc == 0), stop=(dc == DC - 1),
                        )
                        nc.tensor.matmul(
                            v_ps, wvc[:, dc % KV_DC, :], ht,
                            start=(dc == 0), stop=(dc == DC - 1),
                        )
                    # previous block's tail, spread over this block's stream
                    if pending_tail and hc >= 1 and (hc % 2 == 1):
                        pending_tail.pop(0)()
                while pending_tail:
                    pending_tail.pop(0)()
                pending_tail = make_tail(sb, q_ps, k_ps, v_ps, sqacc, sqacc2)

            ph1.close()  # free accumulation PSUM banks + DMA/weight SBUF

            # ------------- Phase 3+4 interleaved ------------------------------
            ph3 = ExitStack()
            sc_ps_p = ph3.enter_context(
                tc.tile_pool(name="sc_ps", bufs=KNOBS["sc_bufs"], space="PSUM")
            )
            att_ps_p = ph3.enter_context(
                tc.tile_pool(name="att_ps", bufs=KNOBS.get("att_bufs", 1), space="PSUM")
            )
            sum_ps_p = ph3.enter_context(
                tc.tile_pool(name="sum_ps", bufs=1, space="PSUM")
            )
            expp = ph3.enter_context(tc.tile_pool(name="expp", bufs=KNOBS["expp_bufs"]))
            scr3 = ph3.enter_context(tc.tile_pool(name="scr3", bufs=2))
            tail_stack = ExitStack()
            tail_psum[0] = tail_stack.enter_context(
                tc.tile_pool(name="tailp", bufs=2, space="PSUM"))
            o_ps_holder = [None]

            def emit_attention_head(sb, h):
                """scores + exp + mask + attnv + eacc chain for one head,
                chunk-at-a-time (single-bank score tiles, deep pipeline).
                Returns a finalize closure to call one head later."""
                ssl = slice(SB * sb, SB * (sb + 1))
                n_tc = (SB // 128) * (sb + 1)
                att_ps = att_ps_p.tile([128, SB], F32, tag="att",
                                       name=f"att{h}_{sb}")
                eacc = scr3.tile([128, SB], BF16, tag="eacc",
                                 bufs=KNOBS.get("eacc_bufs", 2),
                                 name=f"eacc{h}_{sb}")
                # Process the 4 diagonal chunks FIRST (r0 writes att_ps
                # full-width with start=True), then the full chunks; the
                # last full chunk closes every PSUM byte with stop=True.
                # Diagonal chunks r>0 restrict scores/exp/attnv/eacc to the
                # live columns [128r:]; only the true diagonal 128-col
                # sub-block needs the 0/1 mask. For s-block 0 (no full
                # chunks) the final r3 chunk runs untrimmed so its
                # full-width mask zeroes the stale columns and its stop
                # closes the group.
                diag0 = (SB // 128) * sb
                seq = list(range(diag0, n_tc)) + list(range(diag0))
                for idx, tcx in enumerate(seq):
                    r = tcx - diag0
                    first = idx == 0
                    last = idx == n_tc - 1
                    full_override = last and r >= 0
                    lo = (128 * r if (KNOBS.get("score_trim", True) and r > 0
                                      and not full_override) else 0)
                    sc_ps = sc_ps_p.tile([128, SB], F32, tag="sc",
                                         name=f"sc{h}_{sb}_{tcx}")
                    e_t = expp.tile([128, SB], BF16, tag="e",
                                    name=f"e{h}_{sb}_{tcx}")
                    nc.tensor.matmul(
                        sc_ps[:, lo:],
                        kT_all[:, 128 * tcx: 128 * (tcx + 1)],
                        qT_all[:, h, SB * sb + lo: SB * (sb + 1)],
                        start=True, stop=True,
                    )
                    nc.scalar.activation(
                        out=e_t[:, lo:], in_=sc_ps[:, lo:],
                        func=ACTF.Exp, scale=SM_SCALE
                    )
                    if r >= 0:
                        if full_override:
                            nc.vector.tensor_tensor(
                                e_t, e_t, mask_t[:, r, :], ALU.mult
                            )
                        else:
                            dsl = slice(128 * r, 128 * (r + 1))
                            nc.vector.tensor_tensor(
                                e_t[:, dsl], e_t[:, dsl], mask_t[:, r, dsl],
                                ALU.mult
                            )
                    nc.tensor.matmul(
                        att_ps[:, lo:], v_nat[:, tcx, :], e_t[:, lo:],
                        start=first, stop=last,
                    )
                    if first:
                        nc.vector.tensor_copy(out=eacc, in_=e_t)
                    else:
                        nc.vector.tensor_tensor(eacc[:, lo:], eacc[:, lo:],
                                                e_t[:, lo:], ALU.add)
                (nc.scalar.copy if KNOBS.get("evac_act", True)
                 else nc.vector.tensor_copy)(out=attnT[:, h, ssl], in_=att_ps)

                def finalize():
                    sum_ps = sum_ps_p.tile([1, SB], F32, tag="sumrc",
                                           name=f"sum{h}_{sb}")
                    nc.tensor.matmul(sum_ps, onecb_t, eacc, start=True, stop=True)
                    rcv = scr3.tile([1, SB], F32R, tag="rcv", bufs=2,
                                    name=f"rcv{h}_{sb}")
                    with nc.allow_low_precision(reason="softmax recip row"):
                        nc.vector.reciprocal(out=rcv, in_=sum_ps.bitcast(F32R))
                    rc_ps = sum_ps_p.tile([128, SB], F32, tag="sumrc",
                                          name=f"rc{h}_{sb}")
                    nc.tensor.matmul(rc_ps, oner_t, rcv, start=True, stop=True)
                    rc_sb = scr3.tile([128, SB], BF16, tag="rcsb", bufs=2,
                                      name=f"rcsb{h}_{sb}")
                    (nc.scalar.copy if KNOBS.get("evac_act", True)
                     else nc.vector.tensor_copy)(out=rc_sb, in_=rc_ps)
                    # normalize on Pool (SBUF-only) — keeps DVE on eacc chains
                    (nc.gpsimd if KNOBS.get("norm_pool", True)
                     else nc.vector).tensor_tensor(
                        attnT[:, h, ssl], attnT[:, h, ssl], rc_sb, ALU.mult
                    )

                return finalize

            def emit_outproj_sc(sc, last=False, drain=False):
                """outproj for one 128-row s-chunk: 8 j-tiles of 4 accumulated
                head matmuls, staged to one [128, D] bf16 tile. Copies on DVE
                while attention runs (ACT exps are latency-critical);
                alternate DVE/ACT in the pure-outproj drain. Per-jt DMAs when
                `last` keep the final drain short."""
                o_big = outb.tile([128, D], BF16, tag="obig", name=f"ob{sc}")
                for jt in range(D // SB):
                    jsl = slice(SB * jt, SB * (jt + 1))
                    woc = woc_cache[jt]
                    if drain and KNOBS.get("drain_mix", True):
                        pool, tag = ((o_ps_holder[0], "o"), (att_ps_p, "att"),
                                     (o_ps_holder[0], "o"), (sum_ps_p, "sumrc"))[jt % 4]
                    else:
                        pool, tag = o_ps_holder[0], "o"
                    o_ps = pool.tile([128, SB], F32, tag=tag,
                                     name=f"o{jt}_{sc}")
                    for h in range(QH):
                        nc.tensor.matmul(
                            o_ps,
                            attnT[:, h, 128 * sc: 128 * (sc + 1)],
                            woc[:, h, :],
                            start=(h == 0), stop=(h == QH - 1),
                        )
                    if (drain or not KNOBS.get("obig_dve", True)) and jt % 2 == 1:
                        nc.scalar.copy(out=o_big[:, jsl], in_=o_ps)
                    else:
                        nc.vector.tensor_copy(out=o_big[:, jsl], in_=o_ps)
                    if last:
                        nc.sync.dma_start(out=out4[:, sc, jsl], in_=o_big[:, jsl])
                if not last:
                    nc.sync.dma_start(out=out4[:, sc, :], in_=o_big)

            # interleave: sb3's phase-1 tail within the first attention heads
            # (2 closures per head); outproj s-chunks of earlier blocks
            # between heads; head finalization lags one head.
            pending_fin = []
            pending_sc = []

            def pump_fin():
                while pending_fin:
                    pending_fin.pop(0)()

            for sb in range(NSB):
                for h in range(QH):
                    fin = emit_attention_head(sb, h)
                    pump_fin()
                    pending_fin.append(fin)
                    consumed = False
                    for _ in range(2):
                        if pending_tail:
                            pending_tail.pop(0)()
                            consumed = True
                            if not pending_tail:
                                tail_stack.close()
                                o_ps_holder[0] = ph3.enter_context(
                                    tc.tile_pool(name="o_ps",
                                                 bufs=KNOBS.get("o_ps_bufs", 2),
                                                 space="PSUM"))
                    if not consumed and pending_sc:
                        emit_outproj_sc(pending_sc.pop(0))
                pump_fin()
                pending_sc.extend(range(4 * sb, 4 * sb + 4))
            for i, sc in enumerate(pending_sc):
                emit_outproj_sc(sc, last=(i == len(pending_sc) - 1), drain=True)
            ph3.close()

    if not skip_compile:
        nc.compile()
    return nc


def _host_prep(inputs):
    """Build per-core input maps (shard + transpose + fold norm_w + rope-perm)."""
    import ml_dtypes
    BF = ml_dtypes.bfloat16

    hidden = np.ascontiguousarray(np.asarray(inputs["hidden"], dtype=np.float32))
    norm_w = np.asarray(inputs["norm_w"], dtype=np.float32)
    wq = np.asarray(inputs["wq"], dtype=np.float32)
    wk = np.asarray(inputs["wk"], dtype=np.float32)
    wv = np.asarray(inputs["wv"], dtype=np.float32)
    wo = np.asarray(inputs["wo"], dtype=np.float32)

    perm = np.concatenate([np.arange(0, HD, 2), np.arange(1, HD, 2)])
    freqs = 1.0 / THETA ** (np.arange(0, HD, 2)[: HD // 2].astype(np.float32) / HD)
    ang = np.outer(np.arange(S), freqs).astype(np.float32)   # [S, 64]
    cosT = np.ascontiguousarray(
        np.concatenate([np.cos(ang).T, np.cos(ang).T], axis=0).astype(np.float32)
    )
    sinT = np.ascontiguousarray(
        np.concatenate([np.sin(ang).T, np.sin(ang).T], axis=0).astype(np.float32)
    )
    Pr = np.zeros((HD, HD), np.float32)
    Pr[np.arange(64), np.arange(64) + 64] = -1.0
    Pr[np.arange(64) + 64, np.arange(64)] = 1.0
    protT = np.ascontiguousarray(Pr.T)

    hT = np.ascontiguousarray(hidden.T.astype(BF))
    ident = np.eye(128, dtype=np.float32)
    p_i = np.arange(128)[:, None]
    c_i = np.arange(SB)[None, :]
    maskT = np.concatenate(
        [(128 * r + p_i <= c_i).astype(np.float32) for r in range(4)], axis=1
    )
    maskT = np.ascontiguousarray(maskT.astype(BF))
    ones_col = np.ones((128, 1), np.float32)
    ones_col_bf = np.ones((128, 1), np.float32).astype(BF)
    ones_row = np.ones((1, 128), np.float32)

    in_maps = []
    for c in range(NCORES):
        wq_c = wq[QI * c: QI * (c + 1)].reshape(QH, HD, D)[:, perm, :].reshape(QI, D)
        wqT = np.ascontiguousarray((wq_c * norm_w[None, :]).T.astype(BF))
        wk_c = wk[HD * c: HD * (c + 1)][perm, :]
        wkT = np.ascontiguousarray((wk_c * norm_w[None, :]).T.astype(BF))
        wv_c = wv[HD * c: HD * (c + 1)]
        wvT = np.ascontiguousarray((wv_c * norm_w[None, :]).T.astype(BF))
        woT = np.ascontiguousarray(wo[:, QI * c: QI * (c + 1)].T.astype(BF))
        in_maps.append({
            "hT": hT, "wqT": wqT, "wkT": wkT, "wvT": wvT, "woT": woT,
            "cosT": cosT, "sinT": sinT, "protT": protT, "ident": ident,
            "ones_col": ones_col, "ones_col_bf": ones_col_bf,
            "ones_row": ones_row, "maskT": maskT,
        })
    return in_maps


def kernel(**inputs) -> np.ndarray:
    global LAST_EXEC_NS, LAST_RESULT
    if "nc" not in _CACHE:
        _CACHE["nc"] = _build()
    nc = _CACHE["nc"]
    in_maps = _host_prep(inputs)
    res = run_bass_kernel_spmd(nc, in_maps, core_ids=list(range(NCORES)))
    LAST_RESULT = res
    LAST_EXEC_NS = res.exec_time_ns
    out = res.results[0]["outp"].astype(np.float32)
    for c in range(1, NCORES):
        out += res.results[c]["outp"].astype(np.float32)
    return out


# revision 32
# speedup vs baseline: 1.2242x; 1.0018x over previous
"""Trainium2 Bass kernel for nn_AttentionModule (S=2048, D=4096, H=32, KV=8, HD=128).

Sharding: tensor-parallel over heads across 8 NeuronCores. Core c owns q-heads
4c..4c+3 and kv-head c (GQA groups stay intact). Each core computes RMSNorm
(norm_w folded into weights on host, rstd computed on device), its QKV
projection shard, RoPE, causal attention for its 4 heads, and a partial output
projection against its 512 columns of wo. The host sums the 8 partial outputs
(the "all-reduce" of the tensor-parallel layout).

v3: whole pipeline in bf16 (PSUM accumulation fp32; rstd / softmax-denominator
math fp32). bf16 matmuls cost 1 cycle/row like fp32r but halve DMA + SBUF and
unlock DVE 2x perf modes. Exp row-sums run as DVE bf16 chain adds + one
ones-matmul per (head, s-block). rstd = exp(-0.5 ln(ms)) so every ACT function
(Square/Ln/Exp/Copy) lives in one table set — no mid-kernel table reloads.

Scheduling: each s-block's rstd/RoPE/evacuation tail is emitted *inside* the
next s-block's matmul stream (phase 1) or between the first attention heads
(last block), so the in-order PE queue always has dense matmul work while the
serial ACT/DVE chains resolve. Attention head finalization (sum -> recip ->
broadcast -> normalize) lags one head; output projection is emitted per
128-row s-chunk interleaved between the next s-block's attention heads.
Evacuation copies spread across ACT/DVE/Pool.

Causal handling: scores matmuls for the 4 diagonal t-chunks per s-block are
trimmed to the live column range (bf16 matmuls have no <256-free-dim penalty);
the masked multiply covers the full range and zeroes the stale left part. The
exp for the (r2,r3) diagonal pair is likewise trimmed to columns 256: for
s-blocks >= 1 (for block 0 the e buffers must be written full-width once so
uninitialized SBUF never reaches the mask-multiply).
"""
import sys

sys.path.insert(0, "/opt/trn_rl_repo")

import math
from contextlib import ExitStack

import numpy as np

import bass_rust as _bass_rust
import concourse.bacc as bacc
import concourse.mybir as mybir
import concourse.tile as tile
from concourse.bass_utils import run_bass_kernel_spmd
from concourse.hw_specs import get_activation_tables

F32R = mybir.dt.float32r
F32 = mybir.dt.float32
BF16 = mybir.dt.bfloat16
ALU = mybir.AluOpType
ACTF = mybir.ActivationFunctionType

S, D, H, KV, HD = 2048, 4096, 32, 8, 128
NCORES = 8
QH = H // NCORES          # 4 q heads per core
QI = QH * HD              # 512 local q dims
DC = D // 128             # 32 contraction chunks
SB = 512                  # s-block width
NSB = S // SB             # 4 s-blocks
NTC = S // 128            # 16 t-chunks
EPS = 1e-6
THETA = 50000.0
SM_SCALE = 1.0 / math.sqrt(HD)

LAST_EXEC_NS = None
LAST_RESULT = None
_CACHE = {}

KNOBS = dict(hb_bufs=7, t12_bufs=1, expp_bufs=3, qtmp_bufs=6,
             sc_bufs=4, wkv_bufs=2, sqp_bufs=3, hb_dc=2, kv_dc=4, wq_dc=4,
             wop_bufs=8, outb_bufs=2, o_ps_bufs=2, eacc_bufs=2,
             mask_pool_sb=-1, ham_warmup=7, csb=2,
             score_trim=True, exp_trim=True,
             norm_pool=False, evac_act=True, obig_dve=True)

import os as _os
if _os.environ.get("KNOBS_JSON"):
    import json as _json
    KNOBS.update(_json.loads(_os.environ["KNOBS_JSON"]))


class _Bacc(bacc.Bacc):
    """Bacc with activation tables reordered so the one set containing
    Exp+Ln+Copy+Square is preferred."""

    def insert_act_table_loads(self):
        has_activation = any(
            isinstance(i, mybir.InstActivation)
            for b in self.main_func.blocks
            for i in b.instructions
        )
        if not has_activation:
            return
        tables = list(get_activation_tables(self.m.arch).items())
        tables.sort(key=lambda kv: 0 if kv[0] == "natural_log_exp_and_others" else 1)
        _bass_rust.insert_act_table_loads(self, tables)


def _build(skip_compile=False):
    nc = bacc.Bacc("TRN2", target_bir_lowering=False, debug=False)

    hT_d = nc.dram_tensor("hT", [D, S], BF16, kind="ExternalInput")
    wqT_d = nc.dram_tensor("wqT", [D, QI], BF16, kind="ExternalInput")
    wkT_d = nc.dram_tensor("wkT", [D, HD], BF16, kind="ExternalInput")
    wvT_d = nc.dram_tensor("wvT", [D, HD], BF16, kind="ExternalInput")
    woT_d = nc.dram_tensor("woT", [QI, D], BF16, kind="ExternalInput")
    cos_d = nc.dram_tensor("cosT", [128, S], F32R, kind="ExternalInput")
    sin_d = nc.dram_tensor("sinT", [128, S], F32R, kind="ExternalInput")
    prot_d = nc.dram_tensor("protT", [128, 128], F32R, kind="ExternalInput")
    ident_d = nc.dram_tensor("ident", [128, 128], F32R, kind="ExternalInput")
    onec_d = nc.dram_tensor("ones_col", [128, 1], F32R, kind="ExternalInput")
    onecb_d = nc.dram_tensor("ones_col_bf", [128, 1], BF16, kind="ExternalInput")
    oner_d = nc.dram_tensor("ones_row", [1, 128], F32R, kind="ExternalInput")
    mask_d = nc.dram_tensor("maskT", [128, 4 * SB], BF16, kind="ExternalInput")
    out_d = nc.dram_tensor("outp", [S, D], BF16, kind="ExternalOutput")

    hT3 = hT_d.rearrange("(o p) s -> p o s", p=128)      # [128, 32, 2048]
    wqT3 = wqT_d.rearrange("(o p) i -> p o i", p=128)    # [128, 32, 512]
    wkT3 = wkT_d.rearrange("(o p) e -> p o e", p=128)    # [128, 32, 128]
    wvT3 = wvT_d.rearrange("(o p) e -> p o e", p=128)
    woT3 = woT_d.rearrange("(g p) j -> p g j", p=128)    # [128, 4, 4096]
    out4 = out_d.rearrange("(g p) j -> p g j", p=128)    # [128, 16, 4096]

    HB_DC = KNOBS.get("hb_dc", 2)  # hT chunks per DMA

    with tile.TileContext(nc) as tc:
        with ExitStack() as root:
            consts = root.enter_context(tc.tile_pool(name="consts", bufs=1))
            persist = root.enter_context(tc.tile_pool(name="persist", bufs=1))

            onec_t = consts.tile([128, 1], F32R, tag="onec")
            nc.sync.dma_start(out=onec_t, in_=onec_d[:, :])
            onecb_t = consts.tile([128, 1], BF16, tag="onecb")
            nc.sync.dma_start(out=onecb_t, in_=onecb_d[:, :])
            oner_t = consts.tile([1, 128], F32R, tag="oner")
            nc.sync.dma_start(out=oner_t, in_=oner_d[:, :])
            eps_t = consts.tile([1, 1], F32, tag="eps")
            nc.vector.memset(eps_t, EPS)

            qT_all = persist.tile([128, QH, S], BF16, tag="qT")
            kT_all = persist.tile([128, S], BF16, tag="kT")
            v_nat = persist.tile([128, NTC, HD], BF16, tag="vn")
            attnT = persist.tile([128, QH, S], BF16, tag="attnT")
            mask_t = persist.tile([128, 4, SB], BF16, tag="mask")

            # phase-3/4 SBUF staging allocated up front (fits alongside
            # phase 1; lets wo/mask DMAs run during phase-1 DMA idle)
            outb = root.enter_context(tc.tile_pool(name="outb", bufs=KNOBS.get("outb_bufs", 3)))
            wop = root.enter_context(tc.tile_pool(name="wop", bufs=KNOBS.get("wop_bufs", 8)))
            woc_cache = {}

            def load_woc(jt):
                woc = wop.tile([128, QH, SB], BF16, tag="wo", name=f"wo{jt}")
                nc.sync.dma_start(out=woc, in_=woT3[:, :, SB * jt: SB * (jt + 1)])
                woc_cache[jt] = woc

            # ------------- Phase 1: QKV projections + rstd + RoPE -------------
            # c1/scr live on root: the sb3 tail (emitted between the first
            # attention heads) still needs them in phase 3
            c1 = root.enter_context(tc.tile_pool(name="c1", bufs=1))
            cos_t = c1.tile([128, S], F32R, tag="cos")
            sin_t = c1.tile([128, S], F32R, tag="sin")
            prot_t = c1.tile([128, 128], F32R, tag="prot")
            ident_t = c1.tile([128, 128], F32R, tag="ident")
            c1_loaded = [False]

            scr = root.enter_context(tc.tile_pool(name="scr", bufs=2))
            ph1 = ExitStack()
            wqp = ph1.enter_context(tc.tile_pool(name="wqp", bufs=1))
            wq_t = wqp.tile([128, DC, QI], BF16, tag="wqr")
            wkvp = ph1.enter_context(tc.tile_pool(name="wkvp", bufs=KNOBS["wkv_bufs"]))
            hb = ph1.enter_context(tc.tile_pool(name="hb", bufs=KNOBS["hb_bufs"]))
            sqp = ph1.enter_context(tc.tile_pool(name="sqp", bufs=KNOBS["sqp_bufs"]))
            acc_ps = ph1.enter_context(tc.tile_pool(name="acc_ps", bufs=1, space="PSUM"))
            misc_ps = ph1.enter_context(tc.tile_pool(name="misc_ps", bufs=2, space="PSUM"))
            # tail closures allocate their PSUM through this holder: misc_ps
            # during phase 1, the ph3 tail pool for the last block
            tail_psum = [misc_ps]

            if KNOBS.get("ham_warmup", 0):
                # HAM clock-ramp warm-up: dummy matmuls during initial DMA
                # wait so real matmuls start at 2.4GHz.
                wu_f = scr.tile([128, SB], F32, tag="wuf", bufs=1,
                                name="warmup_f")
                nc.vector.memset(wu_f, 0.0)
                wu = scr.tile([128, SB], F32R, tag="qtmp", bufs=KNOBS["qtmp_bufs"],
                              name="warmup_src")
                with nc.allow_low_precision(reason="warmup zeros"):
                    nc.vector.tensor_copy(out=wu, in_=wu_f)
                wu_ps = misc_ps.tile([128, SB], F32, tag="misc", name="wu_ps")
                for _w in range(KNOBS["ham_warmup"]):
                    nc.tensor.matmul(wu_ps, wu[:, :128], wu,
                                     start=(_w == 0),
                                     stop=(_w == KNOBS["ham_warmup"] - 1))

            def make_tail(sb, q_ps, k_ps, v_ps, sqacc, sqacc2):
                """rstd + rope + v-transpose for s-block sb, split into
                closures emitted later (inside the next block's matmul
                stream). Returns a list of emit-closures.

                For the last block the six accumulator PSUMs are evacuated to
                SBUF right here (ACT copies, no PE involvement) so the
                attention pools can take over every PSUM bank; the deferred
                closures then run RoPE from the SBUF temps."""
                ssl = slice(SB * sb, SB * (sb + 1))
                parts = []
                last = sb == NSB - 1
                tmps = {}
                if last:
                    for nm, ps in [("q0", q_ps[0]), ("q1", q_ps[1]),
                                   ("q2", q_ps[2]), ("q3", q_ps[3]),
                                   ("k", k_ps), ("v", v_ps)]:
                        t = scr.tile([128, SB], F32R, tag="qtmp",
                                     bufs=KNOBS["qtmp_bufs"], name=f"fin_{nm}")
                        with nc.allow_low_precision(reason="rope tmp f32r"):
                            nc.scalar.copy(out=t, in_=ps)
                        tmps[nm] = t

                def p_rstd():
                    # ms = colsum(sqacc) + colsum(sqacc2) via two accumulating
                    # K=1 matmuls; rstd = exp(-0.5 ln(ms/D + eps))
                    ms_ps = tail_psum[0].tile([1, SB], F32, tag="misc", name=f"ms{sb}")
                    nc.tensor.matmul(ms_ps, onec_t, sqacc,
                                     start=True, stop=False)
                    nc.tensor.matmul(ms_ps, onec_t, sqacc2,
                                     start=False, stop=True)
                    lnt = scr.tile([1, SB], F32, tag="lnt", bufs=1, name=f"ln{sb}")
                    nc.scalar.activation(
                        out=lnt, in_=ms_ps, func=ACTF.Sqrt, scale=1.0 / D, bias=eps_t
                    )
                    rstd = scr.tile([1, SB], F32R, tag="rstd", bufs=1, name=f"rst{sb}")
                    with nc.allow_low_precision(reason="rstd row fp32r"):
                        nc.vector.reciprocal(out=rstd, in_=lnt.bitcast(F32R))
                    rb_ps = tail_psum[0].tile([128, SB], F32, tag="misc", name=f"rb{sb}")
                    nc.tensor.matmul(rb_ps, oner_t, rstd,
                                     start=True, stop=True)
                    rb_sb = scr.tile([128, SB], F32R, tag="rb_sb",
                                     bufs=KNOBS.get("csb", 2), name=f"rbs{sb}")
                    nc.vector.tensor_copy(out=rb_sb, in_=rb_ps.bitcast(F32R))
                    cosrb = scr.tile([128, SB], F32R, tag="cosrb",
                                     bufs=KNOBS.get("csb", 2), name=f"cr{sb}")
                    nc.vector.tensor_tensor(cosrb, cos_t[:, ssl], rb_sb, ALU.mult)
                    sinrb = scr.tile([128, SB], F32R, tag="sinrb",
                                     bufs=KNOBS.get("csb", 2), name=f"sr{sb}")
                    nc.vector.tensor_tensor(sinrb, sin_t[:, ssl], rb_sb, ALU.mult)
                    parts.append((rb_sb, cosrb, sinrb))

                def rope_one(dst, src_ps, cosrb, sinrb, nm, tmp=None):
                    # evac on ACT (f32r); rot on PE; t1/t2/add DVE in f32r
                    # (homogeneous dtypes, baseline-proven on hw); the add
                    # writes the bf16 destination directly
                    if tmp is None:
                        tmp = scr.tile([128, SB], F32R, tag="qtmp",
                                       bufs=KNOBS["qtmp_bufs"], name=f"tmp{nm}")
                        with nc.allow_low_precision(reason="rope tmp f32r"):
                            nc.scalar.copy(out=tmp, in_=src_ps)
                    rot_ps = tail_psum[0].tile([128, SB], F32, tag="misc",
                                               name=f"rot{nm}")
                    nc.tensor.matmul(rot_ps, prot_t, tmp, start=True, stop=True)
                    t1 = scr.tile([128, SB], F32R, tag="t1",
                                  bufs=KNOBS["t12_bufs"], name=f"t1{nm}")
                    nc.vector.tensor_tensor(t1, tmp, cosrb, ALU.mult)
                    t2 = scr.tile([128, SB], F32R, tag="t2",
                                  bufs=KNOBS["t12_bufs"], name=f"t2{nm}")
                    nc.vector.tensor_tensor(t2, rot_ps.bitcast(F32R), sinrb, ALU.mult)
                    nc.vector.tensor_tensor(dst, t1, t2, ALU.add)

                def p_k():
                    rb_sb, cosrb, sinrb = parts[0]
                    rope_one(kT_all[:, ssl], k_ps, cosrb, sinrb, f"k{sb}",
                             tmp=tmps.get("k"))

                def p_q(i):
                    def f():
                        rb_sb, cosrb, sinrb = parts[0]
                        rope_one(qT_all[:, i, ssl], q_ps[i], cosrb, sinrb,
                                 f"q{i}_{sb}", tmp=tmps.get(f"q{i}"))
                    return f

                def p_v():
                    rb_sb, cosrb, sinrb = parts[0]
                    vtmp = tmps.get("v")
                    if vtmp is None:
                        vtmp = scr.tile([128, SB], F32R, tag="qtmp",
                                        bufs=KNOBS["qtmp_bufs"], name=f"vtmp{sb}")
                        with nc.allow_low_precision(reason="rope tmp f32r"):
                            nc.scalar.copy(out=vtmp, in_=v_ps)
                    vsc = scr.tile([128, SB], F32R, tag="vsc",
                                   bufs=KNOBS.get("csb", 2), name=f"vsc{sb}")
                    nc.vector.tensor_tensor(vsc, vtmp, rb_sb, ALU.mult)
                    for j in range(SB // 128):
                        tcx = (SB // 128) * sb + j
                        vtr_ps = tail_psum[0].tile([128, 128], F32R, tag="misc",
                                                   name=f"vtr{tcx}")
                        nc.tensor.transpose(
                            vtr_ps, vsc[:, 128 * j: 128 * (j + 1)], ident_t
                        )
                        nc.vector.tensor_copy(out=v_nat[:, tcx, :], in_=vtr_ps)

                return [p_rstd, p_k, p_q(0), p_q(1), p_q(2), p_q(3), p_v]

            pending_tail = []

            for sb in range(NSB):
                ssl = slice(SB * sb, SB * (sb + 1))
                q_ps = [
                    acc_ps.tile([128, SB], F32, tag=f"q{i}", name=f"q_ps{i}")
                    for i in range(QH)
                ]
                k_ps = acc_ps.tile([128, SB], F32, tag="k")
                v_ps = acc_ps.tile([128, SB], F32, tag="v")
                sqacc = scr.tile([128, SB], F32R, tag="sqacc", bufs=2)
                sqacc2 = scr.tile([128, SB], F32R, tag="sqacc2", bufs=2)
                KV_DC = KNOBS.get("kv_dc", 4)  # wk/wv chunk width
                WQ_DC = KNOBS.get("wq_dc", 2)  # wq load width (sb 0)
                for hc in range(DC // HB_DC):
                    ht2 = hb.tile([128, HB_DC, SB], BF16, tag="h")
                    nc.sync.dma_start(out=ht2, in_=hT3[:, HB_DC*hc:HB_DC*(hc+1), ssl])
                    if (HB_DC * hc) % KV_DC == 0:
                        kc0 = HB_DC * hc
                        wkc = wkvp.tile([128, KV_DC, HD], BF16, tag="wk2")
                        nc.sync.dma_start(out=wkc, in_=wkT3[:, kc0:kc0+KV_DC, :])
                        wvc = wkvp.tile([128, KV_DC, HD], BF16, tag="wv2")
                        nc.sync.dma_start(out=wvc, in_=wvT3[:, kc0:kc0+KV_DC, :])
                    for j in range(HB_DC):
                        dc = HB_DC * hc + j
                        ht = ht2[:, j, :]
                        if sb == 0 and dc % WQ_DC == 0:
                            nc.sync.dma_start(out=wq_t[:, dc:dc+WQ_DC, :],
                                              in_=wqT3[:, dc:dc+WQ_DC, :])
                        wqc = wq_t[:, dc, :]
                        if sb == 0 and dc == 8 and not c1_loaded[0]:
                            nc.sync.dma_start(out=cos_t, in_=cos_d[:, :])
                            nc.sync.dma_start(out=sin_t, in_=sin_d[:, :])
                            nc.sync.dma_start(out=prot_t, in_=prot_d[:, :])
                            nc.sync.dma_start(out=ident_t, in_=ident_d[:, :])
                            nc.sync.dma_start(
                                out=mask_t,
                                in_=mask_d.rearrange("p (r s) -> p r s", s=SB))
                            c1_loaded[0] = True
                        if sb >= 2 and dc % 8 == 4:
                            jt = 4 * (sb - 2) + dc // 8
                            load_woc(jt)
                        sq = sqp.tile([128, SB], F32, tag="sq")
                        nc.scalar.activation(out=sq, in_=ht, func=ACTF.Square)
                        # two interleaved f32 accumulation chains:
                        # even dc on DVE, odd dc on GPSIMD
                        acc, eng = ((sqacc, nc.vector) if dc % 2 == 0
                                    else (sqacc2, nc.gpsimd))
                        if dc < 2:
                            eng.tensor_copy(out=acc, in_=sq)
                        else:
                            eng.tensor_tensor(acc, acc, sq, ALU.add)
                        for i in range(QH):
                            nc.tensor.matmul(
                                q_ps[i],
                                wqc[:, 128 * i: 128 * (i + 1)],
                                ht,
                                start=(dc == 0),
                                stop=(dc == DC - 1),
                            )
                        nc.tensor.matmul(
                            k_ps, wkc[:, dc % KV_DC, :], ht,
                            start=(dc == 0), stop=(dc == DC - 1),
                        )
                        nc.tensor.matmul(
                            v_ps, wvc[:, dc % KV_DC, :], ht,
                            start=(dc == 0), stop=(dc == DC - 1),
                        )
                    # previous block's tail, spread over this block's stream
                    if pending_tail and hc >= 1 and (hc % 2 == 1):
                        pending_tail.pop(0)()
                while pending_tail:
                    pending_tail.pop(0)()
                pending_tail = make_tail(sb, q_ps, k_ps, v_ps, sqacc, sqacc2)

            ph1.close()  # free accumulation PSUM banks + DMA/weight SBUF

            # ------------- Phase 3+4 interleaved ------------------------------
            ph3 = ExitStack()
            sc_ps_p = ph3.enter_context(
                tc.tile_pool(name="sc_ps", bufs=KNOBS["sc_bufs"], space="PSUM")
            )
            att_ps_p = ph3.enter_context(
                tc.tile_pool(name="att_ps", bufs=KNOBS.get("att_bufs", 1), space="PSUM")
            )
            sum_ps_p = ph3.enter_context(
                tc.tile_pool(name="sum_ps", bufs=1, space="PSUM")
            )
            expp = ph3.enter_context(tc.tile_pool(name="expp", bufs=KNOBS["expp_bufs"]))
            scr3 = ph3.enter_context(tc.tile_pool(name="scr3", bufs=2))
            tail_stack = ExitStack()
            tail_psum[0] = tail_stack.enter_context(
                tc.tile_pool(name="tailp", bufs=2, space="PSUM"))
            o_ps_holder = [None]

            def emit_attention_head(sb, h):
                """scores + exp + mask + attnv + eacc chain for one head,
                chunk-at-a-time (single-bank score tiles, deep pipeline).
                Returns a finalize closure to call one head later."""
                ssl = slice(SB * sb, SB * (sb + 1))
                n_tc = (SB // 128) * (sb + 1)
                att_ps = att_ps_p.tile([128, SB], F32, tag="att",
                                       name=f"att{h}_{sb}")
                eacc = scr3.tile([128, SB], BF16, tag="eacc",
                                 bufs=KNOBS.get("eacc_bufs", 2),
                                 name=f"eacc{h}_{sb}")
                # Process the 4 diagonal chunks FIRST (r0 writes att_ps
                # full-width with start=True), then the full chunks; the
                # last full chunk closes every PSUM byte with stop=True.
                # Diagonal chunks r>0 restrict scores/exp/attnv/eacc to the
                # live columns [128r:]; only the true diagonal 128-col
                # sub-block needs the 0/1 mask. For s-block 0 (no full
                # chunks) the final r3 chunk runs untrimmed so its
                # full-width mask zeroes the stale columns and its stop
                # closes the group.
                diag0 = (SB // 128) * sb
                seq = list(range(diag0, n_tc)) + list(range(diag0))
                for idx, tcx in enumerate(seq):
                    r = tcx - diag0
                    first = idx == 0
                    last = idx == n_tc - 1
                    full_override = last and r >= 0
                    lo = (128 * r if (KNOBS.get("score_trim", True) and r > 0
                                      and not full_override) else 0)
                    sc_ps = sc_ps_p.tile([128, SB], F32, tag="sc",
                                         name=f"sc{h}_{sb}_{tcx}")
                    e_t = expp.tile([128, SB], BF16, tag="e",
                                    name=f"e{h}_{sb}_{tcx}")
                    nc.tensor.matmul(
                        sc_ps[:, lo:],
                        kT_all[:, 128 * tcx: 128 * (tcx + 1)],
                        qT_all[:, h, SB * sb + lo: SB * (sb + 1)],
                        start=True, stop=True,
                    )
                    nc.scalar.activation(
                        out=e_t[:, lo:], in_=sc_ps[:, lo:],
                        func=ACTF.Exp, scale=SM_SCALE
                    )
                    if r >= 0:
                        if full_override:
                            nc.vector.tensor_tensor(
                                e_t, e_t, mask_t[:, r, :], ALU.mult
                            )
                        else:
                            dsl = slice(128 * r, 128 * (r + 1))
                            nc.vector.tensor_tensor(
                                e_t[:, dsl], e_t[:, dsl], mask_t[:, r, dsl],
                                ALU.mult
                            )
                    nc.tensor.matmul(
                        att_ps[:, lo:], v_nat[:, tcx, :], e_t[:, lo:],
                        start=first, stop=last,
                    )
                    if first:
                        nc.vector.tensor_copy(out=eacc, in_=e_t)
                    else:
                        nc.vector.tensor_tensor(eacc[:, lo:], eacc[:, lo:],
                                                e_t[:, lo:], ALU.add)
                (nc.scalar.copy if KNOBS.get("evac_act", True)
                 else nc.vector.tensor_copy)(out=attnT[:, h, ssl], in_=att_ps)

                def finalize():
                    sum_ps = sum_ps_p.tile([1, SB], F32, tag="sumrc",
                                           name=f"sum{h}_{sb}")
                    nc.tensor.matmul(sum_ps, onecb_t, eacc, start=True, stop=True)
                    rcv = scr3.tile([1, SB], F32R, tag="rcv", bufs=2,
                                    name=f"rcv{h}_{sb}")
                    with nc.allow_low_precision(reason="softmax recip row"):
                        nc.vector.reciprocal(out=rcv, in_=sum_ps.bitcast(F32R))
                    rc_ps = sum_ps_p.tile([128, SB], F32, tag="sumrc",
                                          name=f"rc{h}_{sb}")
                    nc.tensor.matmul(rc_ps, oner_t, rcv, start=True, stop=True)
                    rc_sb = scr3.tile([128, SB], BF16, tag="rcsb", bufs=2,
                                      name=f"rcsb{h}_{sb}")
                    (nc.scalar.copy if KNOBS.get("evac_act", True)
                     else nc.vector.tensor_copy)(out=rc_sb, in_=rc_ps)
                    # normalize on Pool (SBUF-only) — keeps DVE on eacc chains
                    (nc.gpsimd if KNOBS.get("norm_pool", True)
                     else nc.vector).tensor_tensor(
                        attnT[:, h, ssl], attnT[:, h, ssl], rc_sb, ALU.mult
                    )

                return finalize

            def emit_outproj_sc(sc, last=False, drain=False):
                """outproj for one 128-row s-chunk: 8 j-tiles of 4 accumulated
                head matmuls, staged to one [128, D] bf16 tile. Copies on DVE
                while attention runs (ACT exps are latency-critical);
                alternate DVE/ACT in the pure-outproj drain. Per-jt DMAs when
                `last` keep the final drain short."""
                o_big = outb.tile([128, D], BF16, tag="obig", name=f"ob{sc}")
                for jt in range(D // SB):
                    jsl = slice(SB * jt, SB * (jt + 1))
                    woc = woc_cache[jt]
                    if drain and KNOBS.get("drain_mix", True):
                        pool, tag = ((o_ps_holder[0], "o"), (att_ps_p, "att"),
                                     (o_ps_holder[0], "o"), (sum_ps_p, "sumrc"))[jt % 4]
                    else:
                        pool, tag = o_ps_holder[0], "o"
                    o_ps = pool.tile([128, SB], F32, tag=tag,
                                     name=f"o{jt}_{sc}")
                    for h in range(QH):
                        nc.tensor.matmul(
                            o_ps,
                            attnT[:, h, 128 * sc: 128 * (sc + 1)],
                            woc[:, h, :],
                            start=(h == 0), stop=(h == QH - 1),
                        )
                    if (drain or not KNOBS.get("obig_dve", True)) and jt % 2 == 1:
                        nc.scalar.copy(out=o_big[:, jsl], in_=o_ps)
                    else:
                        nc.vector.tensor_copy(out=o_big[:, jsl], in_=o_ps)
                    if last:
                        nc.sync.dma_start(out=out4[:, sc, jsl], in_=o_big[:, jsl])
                if not last:
                    nc.sync.dma_start(out=out4[:, sc, :], in_=o_big)

            # interleave: sb3's phase-1 tail within the first attention heads
            # (2 closures per head); outproj s-chunks of earlier blocks
            # between heads; head finalization lags one head.
            pending_fin = []
            pending_sc = []

            def pump_fin():
                while pending_fin:
                    pending_fin.pop(0)()

            for sb in range(NSB):
                for h in range(QH):
                    fin = emit_attention_head(sb, h)
                    pump_fin()
                    pending_fin.append(fin)
                    consumed = False
                    for _ in range(KNOBS.get("tail_rate", 2)):
                        if pending_tail:
                            pending_tail.pop(0)()
                            consumed = True
                            if not pending_tail:
                                tail_stack.close()
                                o_ps_holder[0] = ph3.enter_context(
                                    tc.tile_pool(name="o_ps",
                                                 bufs=KNOBS.get("o_ps_bufs", 2),
                                                 space="PSUM"))
                    if not consumed and pending_sc:
                        emit_outproj_sc(pending_sc.pop(0))
                pump_fin()
                pending_sc.extend(range(4 * sb, 4 * sb + 4))
            for i, sc in enumerate(pending_sc):
                emit_outproj_sc(sc, last=(i == len(pending_sc) - 1), drain=True)
            ph3.close()

    if not skip_compile:
        nc.compile()
    return nc


def _host_prep(inputs):
    """Build per-core input maps (shard + transpose + fold norm_w + rope-perm)."""
    import ml_dtypes
    BF = ml_dtypes.bfloat16

    hidden = np.ascontiguousarray(np.asarray(inputs["hidden"], dtype=np.float32))
    norm_w = np.asarray(inputs["norm_w"], dtype=np.float32)
    wq = np.asarray(inputs["wq"], dtype=np.float32)
    wk = np.asarray(inputs["wk"], dtype=np.float32)
    wv = np.asarray(inputs["wv"], dtype=np.float32)
    wo = np.asarray(inputs["wo"], dtype=np.float32)

    perm = np.concatenate([np.arange(0, HD, 2), np.arange(1, HD, 2)])
    freqs = 1.0 / THETA ** (np.arange(0, HD, 2)[: HD // 2].astype(np.float32) / HD)
    ang = np.outer(np.arange(S), freqs).astype(np.float32)   # [S, 64]
    cosT = np.ascontiguousarray(
        np.concatenate([np.cos(ang).T, np.cos(ang).T], axis=0).astype(np.float32)
    )
    sinT = np.ascontiguousarray(
        np.concatenate([np.sin(ang).T, np.sin(ang).T], axis=0).astype(np.float32)
    )
    Pr = np.zeros((HD, HD), np.float32)
    Pr[np.arange(64), np.arange(64) + 64] = -1.0
    Pr[np.arange(64) + 64, np.arange(64)] = 1.0
    protT = np.ascontiguousarray(Pr.T)

    hT = np.ascontiguousarray(hidden.T.astype(BF))
    ident = np.eye(128, dtype=np.float32)
    p_i = np.arange(128)[:, None]
    c_i = np.arange(SB)[None, :]
    maskT = np.concatenate(
        [(128 * r + p_i <= c_i).astype(np.float32) for r in range(4)], axis=1
    )
    maskT = np.ascontiguousarray(maskT.astype(BF))
    ones_col = np.ones((128, 1), np.float32)
    ones_col_bf = np.ones((128, 1), np.float32).astype(BF)
    ones_row = np.ones((1, 128), np.float32)

    in_maps = []
    for c in range(NCORES):
        wq_c = wq[QI * c: QI * (c + 1)].reshape(QH, HD, D)[:, perm, :].reshape(QI, D)
        wqT = np.ascontiguousarray((wq_c * norm_w[None, :]).T.astype(BF))
        wk_c = wk[HD * c: HD * (c + 1)][perm, :]
        wkT = np.ascontiguousarray((wk_c * norm_w[None, :]).T.astype(BF))
        wv_c = wv[HD * c: HD * (c + 1)]
        wvT = np.ascontiguousarray((wv_c * norm_w[None, :]).T.astype(BF))
        woT = np.ascontiguousarray(wo[:, QI * c: QI * (c + 1)].T.astype(BF))
        in_maps.append({
            "hT": hT, "wqT": wqT, "wkT": wkT, "wvT": wvT, "woT": woT,
            "cosT": cosT, "sinT": sinT, "protT": protT, "ident": ident,
            "ones_col": ones_col, "ones_col_bf": ones_col_bf,
            "ones_row": ones_row, "maskT": maskT,
        })
    return in_maps


def kernel(**inputs) -> np.ndarray:
    global LAST_EXEC_NS, LAST_RESULT
    if "nc" not in _CACHE:
        _CACHE["nc"] = _build()
    nc = _CACHE["nc"]
    in_maps = _host_prep(inputs)
    res = run_bass_kernel_spmd(nc, in_maps, core_ids=list(range(NCORES)))
    LAST_RESULT = res
    LAST_EXEC_NS = res.exec_time_ns
    out = res.results[0]["outp"].astype(np.float32)
    for c in range(1, NCORES):
        out += res.results[c]["outp"].astype(np.float32)
    return out
